# revision 10
# baseline (speedup 1.0000x reference)
"""Trainium2 Bass kernel for FeatureAttentionLayer (GATv2-style feature attention).

Reference math (per batch b, K=128 features, E=208):
    z[e,i,j] = w0[e]*x[b,i] + w1[e]*x[b,j] + lin_b[e]
    escore[b,i,j] = sum_e a[e]*leakyrelu(z[e,i,j], 0.2) + bias[i,j]
    att = softmax_j(escore);  out[b,i] = sigmoid(sum_j att[b,i,j]*x[b,j])

Kernel algebra:
    leakyrelu(z, 0.2) = z - 0.8*min(z, 0)
    sum_e a_e*z_e = c0*x_i + c1*x_j + c2   (rank-1 terms, scalar c0,c1,c2)
    softmax over j is invariant to terms constant in j  ->  drop c0*x_i + c2.
    escore_eff[i,j] = -0.8*sum_e a_e*min(z_e,0) + c1*x_j + bias[i,j]
    Logits are O(5), so softmax runs without the max-subtraction and the
    softmax+bmm collapses to   out = num/den,  num_i = sum_j P_ji x_j,
    den_i = sum_j P_ji  with  P = exp(escore_eff^T).

Mapping (per core, 4 batches; pure data-parallel over batch):
    - PE outer products build W1X[e,j] = w1_e*x_j (bf16) and
      ND[e,i] = -(w0_e*x_i + b_e) (f32), E=208 in two partition chunks
      (128 + 80, zero padded).
    - per (b, i, chunk) one fused op computes the nonlinear tile (bf16):
        DVE  tensor_scalar: (W1X - ND[:,i]) min 0   = min(z,0)
        ACT  activation:  Relu(-W1X + ND[:,i])      = -min(z,0)
      (engine split by FUSE_PATTERN; ACT tiles pair with +0.8a, DVE with
      -0.8a moving columns, so both accumulate -0.8*a_e*min(z_e,0).)
    - PE matmul with STATIONARY zt, moving (+/-0.8)a column (bf16):
      S^T[:, i] += zt.T @ a  accumulated in PSUM [j, i]; bias^T (optional
      fast path skips zero bias) and the c1*x_j term are injected into the
      same accumulation via identity / rank-1 matmuls.
    - P = Exp(S^T) via ACT (PSUM->SBUF, bf16); two k=128 matmuls with
      lhsT = x_b / ones give num/den rows [1, K];
      out row = 1/(1+exp(-num/den)) DMAs straight to DRAM.
"""

import sys

sys.path.insert(0, "/opt/trn_rl_repo")

import numpy as np

import concourse.bacc as bacc
from concourse import mybir
from concourse.bass_utils import run_bass_kernel_spmd
from concourse.tile import TileContext

N_CORES = 8
B, K, E = 32, 128, 208
B_LOC = B // N_CORES
# fused-op engine schedule (cycle of 8): v=DVE, g=GpSimd, a=ScalarE
FUSE_PATTERN = "vvavavva"

_FP32 = mybir.dt.float32
_BF16 = mybir.dt.bfloat16


def build_bass(with_bias=True):
    nc = bacc.Bacc("TRN2", target_bir_lowering=False, debug=False, num_devices=N_CORES)

    # x batches packed on one partition: [1, B_LOC*K]
    x1_ext = nc.dram_tensor("x1", [1, B_LOC * K], _BF16, kind="ExternalInput").ap()
    biasT_ext = nc.dram_tensor("biasT", [K, K], _BF16, kind="ExternalInput").ap()
    ident_ext = nc.dram_tensor("ident", [K, K], _BF16, kind="ExternalInput").ap()
    # [w0_c0 | w0_c1 | w1_c0 | w1_c1] packed along free dim of one partition
    prows_ext = nc.dram_tensor("prows", [1, 4 * K], _BF16, kind="ExternalInput").ap()
    pcols_ext = nc.dram_tensor("pcols", [K, 2], _FP32, kind="ExternalInput").ap()
    a08_ext = nc.dram_tensor("a08", [K, 4], _BF16, kind="ExternalInput").ap()
    c1s_ext = nc.dram_tensor("c1s", [1, 1], _FP32, kind="ExternalInput").ap()
    # [x_b.T | ones] column pairs per batch: [K, 2*B_LOC]
    xo_ext = nc.dram_tensor("xo", [K, 2 * B_LOC], _BF16, kind="ExternalInput").ap()
    y_ext = nc.dram_tensor("y", [B_LOC, K], _FP32, kind="ExternalOutput").ap()

    with TileContext(nc) as tc:
        with (
            tc.tile_pool(name="const", bufs=1) as const,
            tc.tile_pool(name="wd", bufs=2) as wd,
            tc.tile_pool(name="zt", bufs=12) as ztp,
            tc.tile_pool(name="ptile", bufs=2) as ptp,
            tc.tile_pool(name="small", bufs=4) as small,
            tc.tile_pool(name="psum", bufs=2, space="PSUM") as psum,
            tc.tile_pool(name="psum1", bufs=1, space="PSUM") as psum1,
        ):
            x1_sb = const.tile([1, B_LOC * K], _BF16)
            nc.sync.dma_start(out=x1_sb, in_=x1_ext)
            if with_bias:
                biasT_sb = const.tile([K, K], _BF16)
                nc.sync.dma_start(out=biasT_sb, in_=biasT_ext)
                ident_sb = const.tile([K, K], _BF16)
                nc.sync.dma_start(out=ident_sb, in_=ident_ext)
            prows_sb = const.tile([1, 4 * K], _BF16)
            nc.sync.dma_start(out=prows_sb, in_=prows_ext)
            pcols_sb = const.tile([K, 2], _FP32)
            nc.sync.dma_start(out=pcols_sb, in_=pcols_ext)
            a08_sb = const.tile([K, 4], _BF16)
            nc.sync.dma_start(out=a08_sb, in_=a08_ext)
            c1s_sb = const.tile([1, 1], _FP32)
            nc.sync.dma_start(out=c1s_sb, in_=c1s_ext)
            xo_sb = const.tile([K, 2 * B_LOC], _BF16)
            nc.sync.dma_start(out=xo_sb, in_=xo_ext)
            ones_sb = const.tile([1, K], _BF16)
            nc.vector.memset(ones_sb, 1.0)

            fused_idx = 0
            for b in range(B_LOC):
                x_row = x1_sb[0:1, b * K : (b + 1) * K]

                # --- prep outer products for this batch ---
                w1x = []
                w1x_ps = []
                ndmat = []
                for c in range(2):
                    pw = psum.tile([K, K], _FP32, tag="wx")
                    nc.tensor.matmul(
                        pw, prows_sb[0:1, (2 + c) * K : (3 + c) * K], x_row
                    )
                    t = wd.tile([K, K], _BF16, tag=f"w1x{c}")
                    nc.vector.tensor_copy(t, pw)
                    w1x.append(t)
                    w1x_ps.append(pw)

                    pd = psum.tile([K, K], _FP32, tag="d")
                    nc.tensor.matmul(pd, prows_sb[0:1, c * K : (c + 1) * K], x_row)
                    td = wd.tile([K, K], _FP32, tag=f"d{c}")
                    nc.vector.tensor_scalar(
                        out=td,
                        in0=pd,
                        scalar1=pcols_sb[:, c : c + 1],
                        scalar2=-1.0,
                        op0=mybir.AluOpType.add,
                        op1=mybir.AluOpType.mult,
                    )
                    ndmat.append(td)

                # 0.6*c1*x row for the v_j term
                rb = small.tile([1, K], _BF16, tag="rb")
                nc.vector.tensor_scalar(
                    out=rb,
                    in0=x_row,
                    scalar1=c1s_sb[0:1, 0:1],
                    scalar2=None,
                    op0=mybir.AluOpType.mult,
                )

                # --- scores accumulate transposed in PSUM: s_ps[j, i] ---
                s_ps = psum.tile([K, K], _FP32, tag="S")
                if with_bias:
                    nc.tensor.matmul(s_ps, ident_sb, biasT_sb, start=True, stop=False)
                # += c1*x_j along partitions (constant over i)
                nc.tensor.matmul(s_ps, rb, ones_sb, start=not with_bias, stop=False)

                for i in range(K):
                    for c in range(2):
                        zt = ztp.tile([K, K], _BF16, tag="zt")
                        ndcol = ndmat[c][:, i : i + 1]
                        eng = FUSE_PATTERN[fused_idx % len(FUSE_PATTERN)]
                        is_act = eng == "a"
                        if is_act:
                            # relu(-z) = -min(z, 0); pairs with +0.8a moving col
                            nc.scalar.activation(
                                out=zt,
                                in_=w1x[c],
                                func=mybir.ActivationFunctionType.Relu,
                                bias=ndcol,
                                scale=-1.0,
                            )
                        else:
                            # min(z, 0); pairs with -0.8a moving col
                            veng = nc.vector if eng == "v" else nc.gpsimd
                            veng.tensor_scalar(
                                out=zt,
                                in0=w1x[c],
                                scalar1=ndcol,
                                scalar2=0.0,
                                op0=mybir.AluOpType.subtract,
                                op1=mybir.AluOpType.min,
                            )
                        fused_idx += 1
                        acol = 2 * int(is_act) + c
                        # S^T[:, i] += zt.T @ (-/+0.8 a_c)   (zt stationary, k = e)
                        nc.tensor.matmul(
                            s_ps[:, i : i + 1],
                            zt,
                            a08_sb[:, acol : acol + 1],
                            start=False,
                            stop=(c == 1),
                            skip_group_check=True,
                        )

                # --- P = exp(S^T); num/den via one matmul; sigmoid(num/den) ---
                pexp = ptp.tile([K, K], _BF16, tag="pexp")
                nc.scalar.activation(
                    out=pexp,
                    in_=s_ps,
                    func=mybir.ActivationFunctionType.Exp,
                    bias=0.0,
                    scale=1.0,
                )
                num_ps = psum1.tile([1, K], _FP32, tag="ndn")
                nc.tensor.matmul(num_ps, xo_sb[:, 2 * b : 2 * b + 1], pexp)
                den_ps = psum1.tile([1, K], _FP32, tag="ndd")
                nc.tensor.matmul(den_ps, xo_sb[:, 2 * b + 1 : 2 * b + 2], pexp)

                dinv = small.tile([1, K], _FP32, tag="dinv")
                nc.vector.reciprocal(out=dinv, in_=den_ps)
                ratio = small.tile([1, K], _FP32, tag="ratio")
                nc.vector.tensor_tensor(
                    out=ratio, in0=num_ps, in1=dinv, op=mybir.AluOpType.mult
                )
                # sigmoid(r) = 1/(1+exp(-r)) -- stays in the Exp table set
                esig = small.tile([1, K], _FP32, tag="esig")
                nc.scalar.activation(
                    out=esig,
                    in_=ratio,
                    func=mybir.ActivationFunctionType.Exp,
                    bias=0.0,
                    scale=-1.0,
                )
                sig1 = small.tile([1, K], _FP32, tag="sig1")
                nc.vector.tensor_scalar(
                    out=sig1,
                    in0=esig,
                    scalar1=1.0,
                    scalar2=None,
                    op0=mybir.AluOpType.add,
                )
                resrow = small.tile([1, K], _FP32, tag="resrow")
                nc.vector.reciprocal(out=resrow, in_=sig1)
                nc.sync.dma_start(out=y_ext[b : b + 1, :], in_=resrow)

    nc.compile()
    return nc


def make_in_maps(x, lin_w, lin_b, a, bias):
    w0 = lin_w[:, 0].astype(np.float32)
    w1 = lin_w[:, 1].astype(np.float32)
    a0 = a[:, 0].astype(np.float32)
    bf16 = mybir.dt.np(_BF16)

    prows = np.zeros((1, 4 * K), np.float32)
    prows[0, 0:128] = w0[:128]
    prows[0, K : K + (E - 128)] = w0[128:]
    prows[0, 2 * K : 2 * K + 128] = w1[:128]
    prows[0, 3 * K : 3 * K + (E - 128)] = w1[128:]

    pcols = np.zeros((K, 2), np.float32)
    pcols[:128, 0] = lin_b[:128]
    pcols[: E - 128, 1] = lin_b[128:]

    a08 = np.zeros((K, 4), np.float32)
    a08[:128, 0] = -0.8 * a0[:128]
    a08[: E - 128, 1] = -0.8 * a0[128:]
    a08[:, 2:4] = -a08[:, 0:2]
    a08 = a08.astype(bf16)

    c1s = np.array([[float(np.dot(a0, w1))]], np.float32)
    ident = np.eye(K, dtype=np.float32).astype(bf16)
    biasT = np.ascontiguousarray(bias.T).astype(bf16)
    prows = prows.astype(bf16)

    in_maps = []
    for core in range(N_CORES):
        xs = x[core * B_LOC : (core + 1) * B_LOC].astype(np.float32)
        xo = np.zeros((K, 2 * B_LOC), np.float32)
        for b in range(B_LOC):
            xo[:, 2 * b] = xs[b]
            xo[:, 2 * b + 1] = 1.0
        in_maps.append(
            {
                "x1": np.ascontiguousarray(xs.reshape(1, B_LOC * K)).astype(bf16),
                "biasT": biasT,
                "ident": ident,
                "prows": prows,
                "pcols": pcols,
                "a08": a08,
                "c1s": c1s,
                "xo": xo.astype(bf16),
            }
        )
    return in_maps


_NC = {}


def _get_nc(with_bias):
    if with_bias not in _NC:
        _NC[with_bias] = build_bass(with_bias)
    return _NC[with_bias]


def kernel(x, lin_w, lin_b, a, bias, _trace=False, _tmpdir=None):
    nc = _get_nc(bool(np.any(np.asarray(bias))))
    in_maps = make_in_maps(
        np.asarray(x), np.asarray(lin_w), np.asarray(lin_b), np.asarray(a), np.asarray(bias)
    )
    res = run_bass_kernel_spmd(
        nc, in_maps, list(range(N_CORES)), trace=_trace, tmpdir=_tmpdir
    )
    out = np.concatenate([res.results[i]["y"] for i in range(N_CORES)], axis=0)
    if _trace:
        return out.astype(np.float32), res
    return out.astype(np.float32)


if __name__ == "__main__":
    rng = np.random.default_rng(0)
    x = rng.standard_normal((B, K), dtype=np.float32)
    lin_w = (rng.standard_normal((E, 2)) * 0.5).astype(np.float32)
    lin_b = (rng.standard_normal((E,)) * 0.1).astype(np.float32)
    a = (rng.standard_normal((E, 1)) * 0.1).astype(np.float32)
    bias = np.zeros((K, K), np.float32)
    out = kernel(x, lin_w, lin_b, a, bias)
    print("out", out.shape, out.dtype, out[0, :4])
